# revision 40
# baseline (speedup 1.0000x reference)
"""CAM (channel-attention) kernel for Trainium2, data-parallel over batch on 8 cores.

Reference computation (per sample b):
    avg[c] = mean over spatial of x[b, c, :, :]
    mx[c]  = max  over spatial of x[b, c, :, :]
    gate   = sigmoid(W2 @ relu(W1 @ avg) + W2 @ relu(W1 @ mx))
    y[b]   = x[b] * gate[:, None, None]

Design (per core, 4 samples; CoreSim v1 cost model):
  - All bulk I/O in fp16: the host pre-casts x to fp16 and up-casts y back
    to f32. A [128, 3136] fp16 tile transfer costs 2418ns of queue time
    (free bytes x 0.3855), half of f32, and DVE tensor_scalar on 2-byte
    packed SBUF operands runs in the 4x perf mode. fp16 round-trip error
    ~0.05%, far inside the 2e-2 gate.
  - Three DMA-capable engines (SP, ACT HWDGE; Pool SWDGE) are the rings; a
    DMA occupies its issuing engine for the transfer cost, so the 32 tile
    transfers (77.4us of ring time) are split ~14/14/4 across SP/ACT/Pool.
  - Stats exploit tensor_scalar's accumulator, whose reduce op follows op1
    (add OR max) and reduces in fp32: one 4x pass per channel group for the
    sum (877ns, op1=add) and one for the max (877ns, op1=max). The Pool
    engine carries no compute: GPSIMD cannot execute vector-ALU opcodes
    (neuron_isa_check rejects TENSOR_TENSOR on Pool), so Pool is a pure
    third DMA ring.
  - The gating multiply is DVE tensor_scalar_mul at 4x (877ns/group);
    ACT_MUL groups instead use ACT activation(Copy, scale=gate) (4181ns),
    interleaved between the per-sample sigmoids, to rebalance DVE's load
    against ACT's idle compute slots. Steady state ~38us on DVE/SP/Pool.
  - The tiny shared MLP runs on TensorE with host-pretransposed weights.
    relu(W1@sum)/S = relu(W1@mean) (positive homogeneity) and W2 is linear,
    so both branches merge into one [32,1] vector before layer 2:
        layer1: psum[32,2] += w1t[:,ci,:].T @ [sum | max], ci=0..3
        hsum = relu(psum)[:,0]*(1/S) + relu(psum)[:,1]
        layer2 per ci: p2[:,ci] = w2t[:,ci*128:].T @ hsum ; gate = sigmoid(p2)
  - A zeroing matmul opens each PSUM accumulation group and warm-up matmuls
    touch the weight tiles once: every hot instruction then needs at most
    one semaphore wait (this toolchain allows exactly ONE wait slot per
    hardware instruction). The gate is copied (g2) on DVE so the muls wait
    on the DMA sem only. nc.compile() legalizes any remaining multi-wait
    instructions into EventSemaphore prefixes.
"""

import numpy as np

import concourse.bacc as bacc
import concourse.bass as bass
import concourse.tile as tile
from concourse import mybir
from concourse.alu_op_type import AluOpType

N_CORES = 8
B = 32
C = 512
S = 56 * 56  # 3136
H1 = S // 2  # 1568
H2 = S // 4  # 784
BPC = B // N_CORES  # samples per core
P = 128
CI = C // P  # channel groups of 128
HID = 32

F32 = mybir.dt.float32
F16 = mybir.dt.float16
AF = mybir.ActivationFunctionType

LAST_RESULTS = None  # BassKernelResults of the most recent run (for test harness)
_NC_CACHE = None

# Groups whose gating multiply runs on ACT (activation Copy with scale):
# two for early samples (ACT is otherwise idle between sigmoids), one for
# the late ones (their ACT muls would otherwise push the drain).
ACT_MUL = ((0, 1), (0, 1), (0, 1), ())

# Per-(b, ci) ring assignment for loads and stores (SP ~15.5, Pool ~15.5,
# ACT ~1 transfers; ACT spends its time on sigmoids + ACT_MUL gating).
# b=0 is hand-placed on SP/Pool: ACT opens with two activation-table loads.
LOAD_ENG = [
    None,                                   # b=0 emitted explicitly
    ("scalar", "sync", "scalar", "sync"),
    ("scalar", "gpsimd", "sync", "gpsimd"),
    ("gpsimd", "sync", "gpsimd", "sync"),
]
STORE_ENG = [
    ("sync", "gpsimd", "sync", "gpsimd"),
    ("gpsimd", "sync", "gpsimd", "sync"),
    ("sync", "gpsimd", "scalar", "gpsimd"),
    None,                                   # b=3 stores are halved below
]

# stats emission order for b0 matches load-landing order (g0 split early)
B0_ORDER = (0, 2, 1, 3)


def _build_bass():
    nc = bacc.Bacc()
    x = nc.dram_tensor("x", (BPC, CI, P, S), F16, kind="ExternalInput")
    w1t = nc.dram_tensor("w1t", (P, CI, HID), F32, kind="ExternalInput")
    w2t = nc.dram_tensor("w2t", (HID, C), F32, kind="ExternalInput")
    y = nc.dram_tensor("y", (BPC, CI, P, S), F16, kind="ExternalOutput")

    def eng(name):
        return getattr(nc, name)

    with tile.TileContext(nc) as tc:
        with (
            tc.tile_pool(name="xp", bufs=4) as xp,
            tc.tile_pool(name="yp", bufs=10) as yp,
            tc.tile_pool(name="dump", bufs=1) as dump,
            tc.tile_pool(name="consts", bufs=1) as consts,
            tc.tile_pool(name="small", bufs=4) as small,
            tc.tile_pool(name="ps1", bufs=4, space=bass.MemorySpace.PSUM) as ps1,
            tc.tile_pool(name="ps2", bufs=4, space=bass.MemorySpace.PSUM) as ps2,
        ):
            xts = [None] * BPC
            xts[0] = xp.tile([P, CI, S], F16, tag="xt", name="xt0")
            # b0 g0 lands in two halves on two rings so DVE can start ~1.2us
            # earlier; all b0 loads avoid the table-load-delayed ACT ring.
            nc.sync.dma_start(out=xts[0][:, 0, 0:H1], in_=x[0, 0][:, 0:H1])
            nc.gpsimd.dma_start(out=xts[0][:, 0, H1:S], in_=x[0, 0][:, H1:S])
            nc.gpsimd.dma_start(out=xts[0][:, 2, :], in_=x[0, 2])
            nc.sync.dma_start(out=xts[0][:, 1, :], in_=x[0, 1])
            # g3 is the last to land and bounds stats(0): split it too
            nc.sync.dma_start(out=xts[0][:, 3, 0:H1], in_=x[0, 3][:, 0:H1])
            nc.gpsimd.dma_start(out=xts[0][:, 3, H1:S], in_=x[0, 3][:, H1:S])

            w1t_sb = consts.tile([P, CI, HID], F32)
            nc.gpsimd.dma_start(out=w1t_sb[:], in_=w1t[:])
            w2t_sb = consts.tile([HID, C], F32)
            nc.sync.dma_start(out=w2t_sb[:], in_=w2t[:])
            zeros = consts.tile([P, CI], F32)
            nc.vector.memset(zeros[:], 0.0)

            # PE observes the two weight-DMA semaphores here, once.
            pw = ps1.tile([HID, 2], F32, tag="p1", name="pw")
            nc.tensor.matmul(pw[:, 0:1], w1t_sb[:, 0, :], w1t_sb[:, 0, 0:1])
            pw2 = ps2.tile([P, CI], F32, tag="p2", name="pw2")
            nc.tensor.matmul(pw2[:, 0:1], w2t_sb[:, 0:P], w2t_sb[:, 0:1])
            # warm the sigmoid activation table off the critical path: the
            # only ACT compute in the hot loop is Sigmoid, so exactly one
            # table set is loaded, here.
            gw = consts.tile([P, CI], F32)
            nc.scalar.activation(out=gw[:], in_=zeros[:], func=AF.Sigmoid)

            def loads(b, xt):
                for ci in range(CI):
                    eng(LOAD_ENG[b][ci]).dma_start(out=xt[:, ci, :], in_=x[b, ci])

            xts[1] = xp.tile([P, CI, S], F16, tag="xt", name="xt1")
            loads(1, xts[1])

            statss = [None] * BPC
            dmys = [None] * BPC

            def alloc_stat_tiles(b):
                statss[b] = small.tile([P, CI, 2], F32, tag="stats", name=f"st{b}")
                dmys[b] = dump.tile([P, 2 * S], F16, tag="dmy", name=f"dmy{b}")

            def emit_sums(b, cis):
                # DVE: fp32-accumulated sum of the raw tile (4x pass)
                for ci in cis:
                    nc.vector.tensor_scalar(
                        out=dmys[b][:, 0:S],
                        in0=xts[b][:, ci, :],
                        scalar1=1.0,
                        scalar2=None,
                        op0=AluOpType.mult,
                        op1=AluOpType.add,
                        accum_out=statss[b][:, ci, 0:1],
                    )

            def emit_maxes(b, cis):
                # DVE: max via tensor_scalar's op1-driven accumulator
                for ci in cis:
                    nc.vector.tensor_scalar(
                        out=dmys[b][:, S : 2 * S],
                        in0=xts[b][:, ci, :],
                        scalar1=1.0,
                        scalar2=None,
                        op0=AluOpType.mult,
                        op1=AluOpType.max,
                        accum_out=statss[b][:, ci, 1:2],
                    )

            alloc_stat_tiles(0)
            emit_sums(0, B0_ORDER)
            emit_maxes(0, B0_ORDER)

            for b in range(BPC):
                xt = xts[b]
                stats = statss[b]

                # layer 1: psum [32, 2] = sum_ci W1[:, ci-block] @ [sum | max]
                p1 = ps1.tile([HID, 2], F32, tag="p1", name=f"p1_{b}")
                nc.tensor.matmul(
                    p1[:], w1t_sb[:, 0, :], zeros[:, 0:2], start=True, stop=False
                )
                for ci in range(CI):
                    nc.tensor.matmul(
                        p1[:],
                        w1t_sb[:, ci, :],
                        stats[:, ci, :],
                        start=False,
                        stop=(ci == CI - 1),
                    )
                # relu on DVE (tensor_scalar max with 0) keeps ACT's
                # activation table pinned to Sigmoid
                h = small.tile([HID, 2], F32, tag="h", name=f"h{b}")
                nc.vector.tensor_scalar_max(out=h[:], in0=p1[:], scalar1=0.0)
                # W2 is linear: merge branches before layer 2.
                # hsum = relu(W1@sum)*(1/S) + relu(W1@max)
                hsum = small.tile([HID, 1], F32, tag="hsum", name=f"hs{b}")
                nc.vector.tensor_scalar(
                    out=hsum[:],
                    in0=h[:, 0:1],
                    scalar1=1.0 / S,
                    scalar2=h[:, 1:2],
                    op0=AluOpType.mult,
                    op1=AluOpType.add,
                )

                # layer 2: four matmuls into disjoint columns of one PSUM bank
                p2 = ps2.tile([P, CI], F32, tag="p2", name=f"p2_{b}")
                nc.tensor.matmul(
                    p2[:],
                    w2t_sb[:, 0:P],
                    zeros[:HID, 0:CI],
                    start=True,
                    stop=False,
                    skip_group_check=True,
                )
                for ci in range(CI):
                    nc.tensor.matmul(
                        p2[:, ci : ci + 1],
                        w2t_sb[:, ci * P : (ci + 1) * P],
                        hsum[:],
                        start=False,
                        stop=(ci == CI - 1),
                        skip_group_check=True,
                    )
                g = small.tile([P, CI], F32, tag="g", name=f"g{b}")
                nc.scalar.activation(out=g[:], in_=p2[:], func=AF.Sigmoid)

                # overlap the PE->ACT round-trip latency with one of the
                # next sample's sums
                if b + 1 < BPC:
                    alloc_stat_tiles(b + 1)
                    emit_sums(b + 1, [0])

                # single-producer copy: the muls then wait on the DMA sem
                # only. b3's muls skip it and read g directly — one less
                # copy + sem hop on the drain-critical sigmoid(3) chain.
                if b == BPC - 1:
                    g2 = g
                else:
                    g2 = small.tile([P, CI], F32, tag="g2", name=f"g2_{b}")
                    nc.vector.tensor_copy(out=g2[:], in_=g[:])

                # prefetch sample b+2 before this sample's muls/stores
                if b + 2 < BPC:
                    xts[b + 2] = xp.tile([P, CI, S], F16, tag="xt", name=f"xt{b + 2}")
                    loads(b + 2, xts[b + 2])

                # ACT is idle at the drain: include it in the rotation
                b3_half_eng = (
                    ("scalar", "sync"),
                    ("gpsimd", "scalar"),
                    ("sync", "gpsimd"),
                    ("scalar", "sync"),
                )
                for ci in range(CI):
                    yt = yp.tile([P, S], F16, tag="yt", name=f"yt{b}_{ci}")
                    if ci in ACT_MUL[b]:
                        # ACT gating mul: activation Copy scaled by its own
                        # sigmoid output (same engine, no extra sem)
                        nc.scalar.activation(
                            out=yt[:],
                            in_=xt[:, ci, :],
                            func=AF.Copy,
                            scale=g[:, ci : ci + 1],
                        )
                        eng(STORE_ENG[b][ci]).dma_start(out=y[b, ci], in_=yt[:])
                    elif b == BPC - 1:
                        # drain: halve the DVE muls/stores so the final bytes
                        # hit free rings ~1.2us after the last mul
                        for hf in range(2):
                            sl = slice(hf * H1, (hf + 1) * H1)
                            nc.vector.tensor_scalar_mul(
                                out=yt[:, sl],
                                in0=xt[:, ci, sl],
                                scalar1=g2[:, ci : ci + 1],
                            )
                            eng(b3_half_eng[ci][hf]).dma_start(
                                out=y[b, ci][:, sl], in_=yt[:, sl]
                            )
                    else:
                        nc.vector.tensor_scalar_mul(
                            out=yt[:],
                            in0=xt[:, ci, :],
                            scalar1=g2[:, ci : ci + 1],
                        )
                        eng(STORE_ENG[b][ci]).dma_start(out=y[b, ci], in_=yt[:])

                if b + 1 < BPC:
                    emit_sums(b + 1, [1, 2, 3])
                    emit_maxes(b + 1, range(CI))
    nc.compile()
    return nc


_RUNNER = None


def _make_runner(nc):
    """jit(shard_map) over the bass_exec custom call — the same lowering
    run_bass_kernel_spmd uses under axon, but built once and cached so
    repeated kernel() calls reuse one loaded executable (loading a second
    copy of the NEFF in the same process wedges the device)."""
    import jax
    from jax.sharding import Mesh, PartitionSpec
    from jax.experimental.shard_map import shard_map
    from concourse.bass2jax import (
        _bass_exec_p,
        install_neuronx_cc_hook,
        partition_id_tensor,
    )

    install_neuronx_cc_hook()
    partition_name = nc.partition_id_tensor.name if nc.partition_id_tensor else None
    in_names, out_names, out_avals = [], [], []
    for alloc in nc.m.functions[0].allocations:
        if not isinstance(alloc, mybir.MemoryLocationSet):
            continue
        name = alloc.memorylocations[0].name
        if alloc.kind == "ExternalInput":
            if name != partition_name:
                in_names.append(name)
        elif alloc.kind == "ExternalOutput":
            out_names.append(name)
            out_avals.append(
                jax.core.ShapedArray(
                    tuple(alloc.tensor_shape), mybir.dt.np(alloc.dtype)
                )
            )
    all_in = in_names + out_names
    if partition_name is not None:
        all_in.append(partition_name)

    def _body(*args):
        operands = list(args)
        if partition_name is not None:
            operands.append(partition_id_tensor())
        outs = _bass_exec_p.bind(
            *operands,
            out_avals=tuple(out_avals),
            in_names=tuple(all_in),
            out_names=tuple(out_names),
            lowering_input_output_aliases=(),
            sim_require_finite=True,
            sim_require_nnan=True,
            nc=nc,
        )
        return tuple(outs)

    devices = jax.devices()[:N_CORES]
    mesh = Mesh(np.asarray(devices), ("core",))
    n_args = len(in_names) + len(out_names)
    fn = jax.jit(
        shard_map(
            _body,
            mesh=mesh,
            in_specs=(PartitionSpec("core"),) * n_args,
            out_specs=(PartitionSpec("core"),) * len(out_names),
            check_rep=False,
        ),
        keep_unused=True,
    )
    assert in_names == ["x", "w1t", "w2t"] and out_names == ["y"], (
        in_names,
        out_names,
    )
    return fn


def kernel(x, w1, w2, **_ignored):
    global _NC_CACHE, _RUNNER
    x = np.asarray(x, dtype=np.float32)
    w1 = np.asarray(w1, dtype=np.float32)  # [HID, C]
    w2 = np.asarray(w2, dtype=np.float32)  # [C, HID]

    # SBUF layouts, pretransposed on host
    w1t = np.ascontiguousarray(
        w1.T.reshape(CI, P, HID).transpose(1, 0, 2)
    )  # [P, CI, HID]; w1t[p, ci, h] = w1[h, ci*128+p]
    w2t = np.ascontiguousarray(w2.T)  # [HID, C]

    if _NC_CACHE is None:
        _NC_CACHE = _build_bass()
    if _RUNNER is None:
        _RUNNER = _make_runner(_NC_CACHE)

    # fp16 device I/O: host pre-casts the input, up-casts the output
    xs = np.ascontiguousarray(
        x.reshape(N_CORES * BPC, CI, P, S).astype(np.float16)
    )
    w1ts = np.concatenate([w1t] * N_CORES, axis=0)
    w2ts = np.concatenate([w2t] * N_CORES, axis=0)
    ybuf = np.zeros_like(xs)
    (y,) = _RUNNER(xs, w1ts, w2ts, ybuf)
    return np.asarray(y).astype(np.float32).reshape(B, C, 56, 56)
